# Initial kernel scaffold
#
"""Trainium2 Bass kernel for nn_MemoryUnit (cosine-sim memory read with sparse
softmax shrinkage), data-parallel over 8 NeuronCores.

Per core (batch shard of 1024 rows):
  phase A : stream memory f32 once; per 128-row tile compute row norms
            (sqrt + one Newton step), write normalized rows transposed into a
            resident fp16 [f, m] operand via DMA-xbar transpose, and write
            (mem * mnorm) fp16 tiles to a DRAM bounce buffer for mm2.
            x is loaded once: row norms + normalized fp16 copy kept resident.
  phase B1: per 128-row batch tile: logits = x_hat @ mhat^T on TensorE (fp16,
            f32 psum), exp on ScalarE (with running row-sum), hard threshold
            v = e * (e > S/M) fused in one DVE op, xbar-transpose v to DRAM.
  phase B2: v^T resident; stream mem*mnorm tiles from bounce buffer once;
            out[b,f] = sum_m v^T[m,b] * nat[m,f], 8 concurrent 1-bank psums;
            evict scaled by 1/sum(v) (softmax S cancels algebraically).

Threshold identity: relu(w-t)*w/(|w-t|+1e-12) == w * 1{w>t} to ~1e-7 rel,
w = e/S, so mask is e > t*S; final L1 norm reduces to division by sum(v).
"""
import sys

sys.path.insert(0, "/opt/trn_rl_repo")

import numpy as np

N_CORES = 8
B_FULL = 8192
B = B_FULL // N_CORES     # 1024 batch rows per core
M = 4000                  # memory rows
MP = 4096                 # padded memory rows
F = 2048                  # features
P = 128
THRESHOLD = 1.0 / M

_CACHE = {}


def build_nc(B=B, M=M, MP=MP, F=F, MC=512, FC=512, dbg=False):
    import concourse.bacc as bacc
    import concourse.mybir as mybir
    import concourse.tile as tile

    fp32 = mybir.dt.float32
    fp16 = mybir.dt.float16
    AF = mybir.ActivationFunctionType
    OP = mybir.AluOpType

    KT = F // P               # k-tiles
    BT = B // P               # batch tiles per core
    MT = MP // P              # padded memory tiles
    HALF_M = MP // 2          # logits psum tile width
    MHT = MT // 2
    MCw = min(MC, HALF_M)
    NMC = HALF_M // MCw
    FCw = min(FC, F)
    NFC = F // FCw
    PAD = float(MP - M)
    thr = 1.0 / M

    nc = bacc.Bacc("TRN2", target_bir_lowering=False, debug=True)
    with tile.TileContext(nc) as tc:
        with tc.tile_pool(name="dram", bufs=1, space="DRAM") as dram:
            xs = dram.tile([B, F], fp32, kind="ExternalInput", uniquify=False, name="xs")
            memory = dram.tile([M, F], fp32, kind="ExternalInput", uniquify=False, name="memory")
            out = dram.tile([B, F], fp32, kind="ExternalOutput", uniquify=False, name="out")
            if dbg:
                natB = dram.tile([MT, P, F], fp16, kind="ExternalOutput", uniquify=False, name="natB")
                vTd = dram.tile([MT, P, B], fp16, kind="ExternalOutput", uniquify=False, name="vTd")
                edbg = dram.tile([BT, P, MP], fp32, kind="ExternalOutput", uniquify=False, name="edbg")
            else:
                natB = nc.dram_tensor("natB", [MT, P, F], fp16)
                vTd = nc.dram_tensor("vTd", [MT, P, B], fp16)

            with tc.tile_pool(name="ps", bufs=2, space="PSUM") as ps, \
                 tc.tile_pool(name="stats", bufs=1) as stats, \
                 tc.tile_pool(name="xres", bufs=1) as xres:

                eps = stats.tile([P, 1], fp32)
                nc.gpsimd.memset(eps[:], 1e-30)
                padc = stats.tile([P, 1], fp32)
                nc.gpsimd.memset(padc[:], -PAD * thr)
                invz = stats.tile([P, BT], fp32)
                invV = stats.tile([P, BT], fp32)

                with tc.tile_pool(name="mhatT_pool", bufs=1) as mhatT_pool:
                    mhatT = mhatT_pool.tile([P, KT, MP], fp16)

                    # phase A + B1 co-resident so B1 matmuls overlap phase-A prep
                    with tc.tile_pool(name="astage", bufs=2) as astage, \
                         tc.tile_pool(name="b1", bufs=1) as b1:
                        # x: norms + normalized fp16 copy (resident)
                        for bt in range(BT):
                            xin = astage.tile([P, F], fp32, tag="fin", bufs=2)
                            nc.sync.dma_start(xin[:], xs[bt * P:(bt + 1) * P, :])
                            zsq = astage.tile([P, 1], fp32, tag="nsq")
                            sqd = astage.tile([P, F], fp16, tag="sqdump", bufs=2)
                            nc.vector.scalar_tensor_tensor(
                                out=sqd[:, :F], in0=xin[:], scalar=1.0, in1=xin[:],
                                op0=OP.bypass, op1=OP.mult, accum_out=zsq[:])
                            s0 = astage.tile([P, 1], fp32, tag="s0")
                            nc.scalar.activation(s0[:], zsq[:], AF.Sqrt, bias=eps[:])
                            iv = invz[:, bt:bt + 1]
                            nc.vector.reciprocal(iv, s0[:])

                        # memory: norms, normalized transpose, nat bounce
                        for mt in range(MT):
                            rows = min(P, M - mt * P)
                            if rows <= 0:
                                mz = astage.tile([P, F], fp16, tag="mhtile", bufs=1)
                                nc.vector.memset(mz[:], 0.0)
                                nc.sync.dma_start_transpose(
                                    mhatT[:, :, mt * P:(mt + 1) * P], mz[:])
                                nc.sync.dma_start(natB[mt], mz[:])
                                continue
                            min_ = astage.tile([P, F], fp32, tag="fin", bufs=2)
                            if rows < P:
                                nc.vector.memset(min_[:], 0.0)
                                nc.sync.dma_start(min_[:rows, :],
                                                  memory[mt * P:mt * P + rows, :])
                            else:
                                nc.sync.dma_start(min_[:],
                                                  memory[mt * P:(mt + 1) * P, :])
                            msq = astage.tile([P, 1], fp32, tag="nsq")
                            sqd = astage.tile([P, F], fp16, tag="sqdump", bufs=2)
                            nc.vector.scalar_tensor_tensor(
                                out=sqd[:, :F], in0=min_[:], scalar=1.0, in1=min_[:],
                                op0=OP.bypass, op1=OP.mult, accum_out=msq[:])
                            s0 = astage.tile([P, 1], fp32, tag="s0")
                            nc.scalar.activation(s0[:], msq[:], AF.Sqrt, bias=eps[:])
                            r0 = astage.tile([P, 1], fp32, tag="r0")
                            nc.vector.reciprocal(r0[:], s0[:])
                            s1 = astage.tile([P, 1], fp32, tag="s1")
                            nc.vector.scalar_tensor_tensor(
                                out=s1[:], in0=msq[:], scalar=r0[:], in1=s0[:],
                                op0=OP.mult, op1=OP.add)  # 2*norm
                            im = astage.tile([P, 1], fp32, tag="im")
                            nc.vector.reciprocal(im[:], s1[:])
                            nc.vector.tensor_scalar_mul(im[:], im[:], 2.0)  # 1/norm
                            mht = astage.tile([P, F], fp16, tag="mhtile", bufs=1)
                            nc.scalar.activation(mht[:], min_[:], AF.Copy, scale=im[:])
                            nc.sync.dma_start_transpose(
                                mhatT[:, :, mt * P:(mt + 1) * P], mht[:])
                            nc.gpsimd.dma_start(natB[mt], min_[:])

                        # ---------------- phase B1 ----------------
                        for bt in range(BT):
                            xin2 = b1.tile([P, F], fp32, tag="xin2", bufs=1)
                            nc.sync.dma_start(xin2[:], xs[bt * P:(bt + 1) * P, :])
                            xh2 = b1.tile([P, F], fp16, tag="xh2", bufs=1)
                            nc.scalar.activation(xh2[:], xin2[:], AF.Copy,
                                                 scale=invz[:, bt:bt + 1])
                            xT = b1.tile([P, KT, P], fp16, tag="xT", bufs=2)
                            nc.sync.dma_start_transpose(xT[:], xh2[:])
                            e = b1.tile([P, MP], fp32, tag="e", bufs=1)
                            sacc = b1.tile([P, 2], fp32, tag="sacc", bufs=2)
                            for half in range(2):
                                lg = ps.tile([P, HALF_M], fp32, tag="big")
                                for k in range(KT):
                                    for mc in range(NMC):
                                        o0 = mc * MCw
                                        nc.tensor.matmul(
                                            lg[:, o0:o0 + MCw],
                                            lhsT=xT[:, k, :],
                                            rhs=mhatT[:, k, half * HALF_M + o0:
                                                      half * HALF_M + o0 + MCw],
                                            start=(k == 0), stop=(k == KT - 1))
                                nc.scalar.activation(
                                    e[:, half * HALF_M:(half + 1) * HALF_M], lg[:],
                                    AF.Exp, accum_out=sacc[:, half:half + 1])
                            if MP > M:
                                nc.vector.memset(e[:, M:MP], 0.0)
                            T = b1.tile([P, 1], fp32, tag="T", bufs=2)
                            # T = ((sacc0 + sacc1) - PAD) * thr
                            nc.vector.tensor_tensor(
                                T[:], sacc[:, 0:1], sacc[:, 1:2], op=OP.add)
                            nc.vector.scalar_tensor_tensor(
                                out=T[:], in0=T[:], scalar=thr, in1=padc[:],
                                op0=OP.mult, op1=OP.add)
                            if dbg:
                                nc.sync.dma_start(edbg[bt], e[:])
                            v = b1.tile([P, MP], fp16, tag="v", bufs=1)
                            vsum = b1.tile([P, 1], fp32, tag="vsum", bufs=2)
                            nc.vector.scalar_tensor_tensor(
                                out=v[:], in0=e[:], scalar=T[:], in1=e[:],
                                op0=OP.is_gt, op1=OP.mult, accum_out=vsum[:])
                            nc.vector.reciprocal(invV[:, bt:bt + 1], vsum[:])
                            for half in range(2):
                                vT = b1.tile([P, MHT, P], fp16, tag="vT", bufs=1)
                                nc.sync.dma_start_transpose(
                                    vT[:], v[:, half * HALF_M:(half + 1) * HALF_M])
                                nc.sync.dma_start(
                                    vTd[half * MHT:(half + 1) * MHT, :,
                                        bt * P:(bt + 1) * P]
                                    .rearrange("t p b -> p t b"), vT[:])

                # ---------------- phase B2 ----------------
                with tc.tile_pool(name="b2", bufs=1) as b2, \
                     tc.tile_pool(name="b2s", bufs=3) as b2s, \
                     tc.tile_pool(name="b2e", bufs=4) as b2e:
                    vTall = b2.tile([P, MT, B], fp16)
                    nc.sync.dma_start(vTall[:, :MT // 2, :],
                                      vTd[:MT // 2, :, :].rearrange("t p b -> p t b"))
                    nc.sync.dma_start(vTall[:, MT // 2:, :],
                                      vTd[MT // 2:, :, :].rearrange("t p b -> p t b"))
                    NSL = HALF_M // FCw     # psum sub-slots per big psum tile
                    for fc in range(NFC):
                        pst = []
                        for i in range((BT + NSL - 1) // NSL):
                            t = ps.tile([P, HALF_M], fp32, tag="big",
                                        name=f"pst{fc}_{i}")
                            pst.append(t)
                        for m in range(MT):
                            nat = b2s.tile([P, FCw], fp16, tag="nat")
                            nc.sync.dma_start(
                                nat[:], natB[m][:, fc * FCw:(fc + 1) * FCw])
                            for bt in range(BT):
                                sl = pst[bt // NSL]
                                j = bt % NSL
                                nc.tensor.matmul(
                                    sl[:, j * FCw:(j + 1) * FCw],
                                    lhsT=vTall[:, m, bt * P:(bt + 1) * P],
                                    rhs=nat[:],
                                    start=(m == 0), stop=(m == MT - 1))
                        for bt in range(BT):
                            sl = pst[bt // NSL]
                            j = bt % NSL
                            ev = b2e.tile([P, FCw], fp32, tag="ev")
                            nc.scalar.activation(
                                ev[:], sl[:, j * FCw:(j + 1) * FCw], AF.Copy,
                                scale=invV[:, bt:bt + 1])
                            nc.sync.dma_start(
                                out[bt * P:(bt + 1) * P, fc * FCw:(fc + 1) * FCw],
                                ev[:])
    nc.compile()
    return nc


def _get_nc():
    if "nc" not in _CACHE:
        _CACHE["nc"] = build_nc()
    return _CACHE["nc"]


def kernel(x: np.ndarray, memory: np.ndarray) -> np.ndarray:
    from concourse.bass_utils import run_bass_kernel_spmd

    x = np.ascontiguousarray(x, dtype=np.float32)
    memory = np.ascontiguousarray(memory, dtype=np.float32)
    nc = _get_nc()
    in_maps = [
        {"xs": x[c * B:(c + 1) * B], "memory": memory} for c in range(N_CORES)
    ]
    res = run_bass_kernel_spmd(nc, in_maps, core_ids=list(range(N_CORES)))
    return np.concatenate([res.results[c]["out"] for c in range(N_CORES)], axis=0)



# revision 6
# speedup vs baseline: 1.2166x; 1.2166x over previous
"""Trainium2 Bass kernel for nn_MemoryUnit (cosine-sim memory read with sparse
softmax shrinkage), data-parallel over 8 NeuronCores.

Single-pass pipelined design (per core, batch shard of 1024 rows):
  x prep   : load x tiles once, row norms, normalized fp16 copy transposed to
             xT (resident, f on partitions) for mm1 lhsT.
  chunked  : memory streamed once in 512-row chunks; per chunk: row norms,
   A+B1      normalized fp16 transpose into a 2-deep SBUF window (mm1 rhs),
             raw fp16 copy to DRAM bounce natB (mm2 rhs); then immediately
             logits psum = xT^T @ win for all 8 batch tiles, evicted as fp16
             LOGITS into a resident [P, 8x4096] buffer.  TensorE starts ~25us
             into the kernel instead of waiting for all of memory.
  thresh   : per batch tile: e = exp(lg) to f32 scratch (accum S), fused
             hard-threshold v = e * (e > S/M) written fp16 in place over the
             logit buffer, then DMA-xbar transpose to resident vT (no DRAM
             bounce).
  B2       : out[b,f] = sum_m vT[m,b] * natB[m,f]; 8 one-bank psums (one per
             batch tile), natB streamed once; evict scaled by 1/sum(v)
             (softmax normalizer cancels algebraically).

Precision: both matmuls fp16 (fp8 would amplify threshold-mask flips past the
error budget: all softmax weights sit near the 1/M threshold).  Storing fp16
LOGITS (not fp16 e) keeps the mask-flip rate at the fp16-matmul level: logit
quantization error ~2.6e-6 abs vs e-quantization ~1.5e-4 logit-equivalent.
Threshold identity: relu(w-t)*w/(|w-t|+1e-12) == w * 1{w>t}, w = e/S, so the
mask is e > S/M; the final L1 norm reduces to division by sum(v).  Padded
memory rows get logit -20 -> e ~ 2e-9: never selected, S unaffected.
"""
import sys

sys.path.insert(0, "/opt/trn_rl_repo")

import numpy as np

N_CORES = 8
B_FULL = 8192
B = B_FULL // N_CORES     # 1024 batch rows per core
M = 4000                  # memory rows
MP = 4096                 # padded memory rows
F = 2048                  # features
P = 128
THRESHOLD = 1.0 / M

_CACHE = {}


def build_nc(B=B, M=M, MP=MP, F=F):
    import concourse.bacc as bacc
    import concourse.mybir as mybir
    import concourse.tile as tile

    fp32 = mybir.dt.float32
    fp16 = mybir.dt.float16
    AF = mybir.ActivationFunctionType
    OP = mybir.AluOpType

    KT = F // P               # 16 k-tiles
    BT = B // P               # 8 batch tiles per core
    MT = MP // P              # 32 padded memory tiles
    CH = 512                  # memory-column chunk width (= 1 psum bank)
    NCH = MP // CH            # 8 chunks
    CPT = CH // P             # 4 memory tiles per chunk
    FCw = 512                 # mm2 feature chunk (= 1 psum bank)
    NFC = F // FCw            # 4
    thr = 1.0 / M

    nc = bacc.Bacc("TRN2", target_bir_lowering=False, debug=True)
    with tile.TileContext(nc) as tc:
        with tc.tile_pool(name="dram", bufs=1, space="DRAM") as dram:
            xs = dram.tile([B, F], fp32, kind="ExternalInput", uniquify=False, name="xs")
            memory = dram.tile([M, F], fp32, kind="ExternalInput", uniquify=False, name="memory")
            out = dram.tile([B, F], fp32, kind="ExternalOutput", uniquify=False, name="out")
            natB = nc.dram_tensor("natB", [MT, P, F], fp16)

            with tc.tile_pool(name="ps", bufs=8, space="PSUM") as ps, \
                 tc.tile_pool(name="stats", bufs=1) as stats, \
                 tc.tile_pool(name="lgp", bufs=1) as lgp:

                eps = stats.tile([P, 1], fp32)
                nc.gpsimd.memset(eps[:], 1e-30)
                invz = stats.tile([P, BT], fp32)
                invV = stats.tile([P, BT], fp32)
                sacc = stats.tile([P, BT], fp32)
                vsum = stats.tile([P, BT], fp32)
                Tt = stats.tile([P, BT], fp32)

                lgt = [lgp.tile([P, MP], fp16, name=f"lg{bt}") for bt in range(BT)]
                for bt in range(BT):
                    nc.vector.memset(lgt[bt][:, M:MP], -20.0)

                # ---------------- phase A + B1 (pipelined) ----------------
                with tc.tile_pool(name="xtp", bufs=1) as xtp, \
                     tc.tile_pool(name="win", bufs=2) as winp, \
                     tc.tile_pool(name="astage", bufs=2) as astage:

                    xTt = [xtp.tile([P, KT, P], fp16, name=f"xT{bt}")
                           for bt in range(BT)]

                    def x_prep(bt):
                        xin = astage.tile([P, F], fp32, tag="xin", bufs=3,
                                          name="xin")
                        nc.sync.dma_start(xin[:], xs[bt * P:(bt + 1) * P, :])
                        zsq = astage.tile([P, 1], fp32, tag="zsq", bufs=2,
                                          name="zsq")
                        sqd = astage.tile([P, F], fp16, tag="sqd", bufs=2,
                                          name="sqd")
                        nc.vector.scalar_tensor_tensor(
                            out=sqd[:, :F], in0=xin[:], scalar=1.0, in1=xin[:],
                            op0=OP.bypass, op1=OP.mult, accum_out=zsq[:])
                        s0 = astage.tile([P, 1], fp32, tag="s0x", bufs=2,
                                         name="s0x")
                        nc.scalar.activation(s0[:], zsq[:], AF.Sqrt, bias=eps[:])
                        nc.vector.reciprocal(invz[:, bt:bt + 1], s0[:])
                        xh = astage.tile([P, F], fp16, tag="xh", bufs=2,
                                         name="xh")
                        nc.scalar.activation(xh[:], xin[:], AF.Copy,
                                             scale=invz[:, bt:bt + 1])
                        nc.sync.dma_start_transpose(xTt[bt][:], xh[:])

                    wins = [None] * NCH

                    def mem_prep(c):
                        w = winp.tile([P, KT, CH], fp16, tag="win", bufs=2,
                                      name=f"win{c}")
                        wins[c] = w
                        for j in range(CPT):
                            mt = c * CPT + j
                            rows = min(P, M - mt * P)
                            min_ = astage.tile([P, F], fp32, tag="mem", bufs=3,
                                               name="memt")
                            if rows < P:
                                nc.vector.memset(min_[:], 0.0)
                                nc.sync.dma_start(
                                    min_[:rows, :],
                                    memory[mt * P:mt * P + rows, :])
                            else:
                                nc.sync.dma_start(
                                    min_[:], memory[mt * P:(mt + 1) * P, :])
                            msq = astage.tile([P, 1], fp32, tag="msq", bufs=2,
                                              name="msq")
                            sqd = astage.tile([P, F], fp16, tag="sqd", bufs=2,
                                              name="sqdm")
                            nc.vector.scalar_tensor_tensor(
                                out=sqd[:, :F], in0=min_[:], scalar=1.0,
                                in1=min_[:], op0=OP.bypass, op1=OP.mult,
                                accum_out=msq[:])
                            s0 = astage.tile([P, 1], fp32, tag="s0m", bufs=2,
                                             name="s0m")
                            nc.scalar.activation(s0[:], msq[:], AF.Sqrt,
                                                 bias=eps[:])
                            r0 = astage.tile([P, 1], fp32, tag="r0", bufs=2,
                                             name="r0")
                            nc.vector.reciprocal(r0[:], s0[:])
                            s1 = astage.tile([P, 1], fp32, tag="s1", bufs=2,
                                             name="s1")
                            nc.vector.scalar_tensor_tensor(
                                out=s1[:], in0=msq[:], scalar=r0[:], in1=s0[:],
                                op0=OP.mult, op1=OP.add)  # 2*norm
                            im = astage.tile([P, 1], fp32, tag="im", bufs=2,
                                             name="im")
                            nc.vector.reciprocal(im[:], s1[:])
                            nc.vector.tensor_scalar_mul(im[:], im[:], 2.0)
                            mht = astage.tile([P, F], fp16, tag="mht", bufs=2,
                                              name="mht")
                            nc.scalar.activation(mht[:], min_[:], AF.Copy,
                                                 scale=im[:])
                            nc.sync.dma_start_transpose(
                                w[:, :, j * P:(j + 1) * P], mht[:])
                            nc.gpsimd.dma_start(natB[mt], min_[:])

                    def mm1(c):
                        w = wins[c]
                        cw = (M - c * CH) if c == NCH - 1 else CH
                        for bt in range(BT):
                            pst = ps.tile([P, CH], fp32, tag="mm",
                                          name=f"ps1_{c}_{bt}")
                            for k in range(KT):
                                nc.tensor.matmul(
                                    pst[:], lhsT=xTt[bt][:, k, :],
                                    rhs=w[:, k, :],
                                    start=(k == 0), stop=(k == KT - 1))
                            nc.scalar.activation(
                                lgt[bt][:, c * CH:c * CH + cw], pst[:, :cw],
                                AF.Copy)

                    # interleaved front: first matmul only needs xT0 + win0;
                    # all x-side ScalarE copies still precede all psum evicts
                    # (no intra-engine circular waits)
                    x_prep(0)
                    mem_prep(0)
                    x_prep(1)
                    mem_prep(1)
                    for bt in range(2, BT):
                        x_prep(bt)
                    for c in range(NCH):
                        if c + 2 < NCH:
                            mem_prep(c + 2)
                        mm1(c)

                with tc.tile_pool(name="vtp", bufs=1) as vtp:
                    vT = vtp.tile([P, MT, BT, P], fp16, name="vT")

                    # ---------------- threshold + transpose ----------------
                    with tc.tile_pool(name="scp", bufs=2) as scp:
                        for bt in range(BT):
                            scratch = scp.tile([P, MP], fp32, tag="sc", bufs=2,
                                               name="scr")
                            nc.scalar.activation(scratch[:], lgt[bt][:], AF.Exp,
                                                 accum_out=sacc[:, bt:bt + 1])
                            nc.vector.tensor_scalar_mul(
                                Tt[:, bt:bt + 1], sacc[:, bt:bt + 1], thr)
                            nc.vector.scalar_tensor_tensor(
                                out=lgt[bt][:], in0=scratch[:],
                                scalar=Tt[:, bt:bt + 1], in1=scratch[:],
                                op0=OP.is_gt, op1=OP.mult,
                                accum_out=vsum[:, bt:bt + 1])
                            nc.vector.reciprocal(invV[:, bt:bt + 1],
                                                 vsum[:, bt:bt + 1])
                            nc.sync.dma_start_transpose(vT[:, :, bt, :],
                                                        lgt[bt][:])

                    # ---------------- B2 ----------------
                    with tc.tile_pool(name="natp", bufs=4) as natp, \
                         tc.tile_pool(name="evp", bufs=4) as evp:
                        for fc in range(NFC):
                            pst = [ps.tile([P, FCw], fp32, tag="mm",
                                           name=f"ps2_{fc}_{bt}")
                                   for bt in range(BT)]
                            for mt in range(MT):
                                nat = natp.tile([P, FCw], fp16, tag="nat", bufs=4,
                                                name="nat")
                                nc.gpsimd.dma_start(
                                    nat[:], natB[mt][:, fc * FCw:(fc + 1) * FCw])
                                for bt in range(BT):
                                    nc.tensor.matmul(
                                        pst[bt][:], lhsT=vT[:, mt, bt, :],
                                        rhs=nat[:],
                                        start=(mt == 0), stop=(mt == MT - 1))
                            for bt in range(BT):
                                ev = evp.tile([P, FCw], fp32, tag="ev", bufs=4,
                                              name="ev")
                                nc.scalar.activation(ev[:], pst[bt][:], AF.Copy,
                                                     scale=invV[:, bt:bt + 1])
                                nc.sync.dma_start(
                                    out[bt * P:(bt + 1) * P,
                                        fc * FCw:(fc + 1) * FCw], ev[:])
    nc.compile()
    return nc


def _get_nc():
    if "nc" not in _CACHE:
        _CACHE["nc"] = build_nc()
    return _CACHE["nc"]


def kernel(x: np.ndarray, memory: np.ndarray) -> np.ndarray:
    from concourse.bass_utils import run_bass_kernel_spmd

    x = np.ascontiguousarray(x, dtype=np.float32)
    memory = np.ascontiguousarray(memory, dtype=np.float32)
    nc = _get_nc()
    in_maps = [
        {"xs": x[c * B:(c + 1) * B], "memory": memory} for c in range(N_CORES)
    ]
    res = run_bass_kernel_spmd(nc, in_maps, core_ids=list(range(N_CORES)))
    return np.concatenate([res.results[c]["out"] for c in range(N_CORES)], axis=0)
